# revision 13
# baseline (speedup 1.0000x reference)
"""Trainium2 Bass kernel for nn_Decoder (attention + LSTM decoder, T=128 steps).

Sharding (8 NeuronCores, one chip):
- Host folds: K' = enc @ (Wk.T @ Wq) * scale so per-step attention needs no q
  matmul; bk folded away (softmax-invariant), bv folded into gate constants.
- Recurrence: tensor-parallel LSTM (each core owns 128 rows of each of the 4
  gates), batch-parallel attention (2 batches/core). Two AllGathers per step
  (ctx and h). States stored doubled (H=2h, C=2c) so sigmoid comes from the
  Tanh table: sig(x) = 0.5*(tanh(x/2)+1); consumers' weights pre-scaled.
- All rank-dependence is carried by per-core input data (one-hot selectors /
  masks), so one identical program runs SPMD on all 8 cores.
- Output projection + log_softmax: sharded over batch (2/core), W_out streamed
  from HBM in bf16, fused logsumexp.
"""
import numpy as np

import concourse.bacc as bacc
import concourse.mybir as mybir
import concourse.tile as tile
from concourse.bass_utils import run_bass_kernel_spmd

F32 = mybir.dt.float32
BF16 = mybir.dt.bfloat16
AF = mybir.ActivationFunctionType
OP = mybir.AluOpType

V, H, S, B, T = 32000, 1024, 512, 16, 128
SOS = 1
NCORES = 8
BLOC = B // NCORES          # 2 batches per core
KT = H // 128               # 8 h-tiles
ST = S // 128               # 4 s-tiles
ET = H // 128               # 8 e-tiles
VCH = 63                    # v-chunks of 512 (padded)
VPAD = VCH * 512            # 32256

_NC_CACHE = {}


def _build_nc(steps, with_sb, with_bout):
    nc = bacc.Bacc("TRN2", target_bir_lowering=False, debug=False,
                   num_devices=NCORES)

    enc_t = nc.dram_tensor("enc_t", [128, ET, BLOC, S], BF16, kind="ExternalInput")
    wkq = nc.dram_tensor("wkq", [128, ET, KT, 128], BF16, kind="ExternalInput")
    wv = nc.dram_tensor("wv", [128, ET, H], BF16, kind="ExternalInput")
    wic = nc.dram_tensor("wic", [128, KT, 4, 128], BF16, kind="ExternalInput")
    whh = nc.dram_tensor("whh", [128, KT, 4, 128], BF16, kind="ExternalInput")
    gch = nc.dram_tensor("gch", [128, 4], F32, kind="ExternalInput")
    h0t = nc.dram_tensor("h0t", [128, KT, B], BF16, kind="ExternalInput")
    c0t = nc.dram_tensor("c0t", [128, B], F32, kind="ExternalInput")
    sel = nc.dram_tensor("sel", [16, BLOC], BF16, kind="ExternalInput")
    hmask = nc.dram_tensor("hmask", [128, BLOC, KT * B], BF16,
                           kind="ExternalInput")
    wout = nc.dram_tensor("wout", [VCH, 128, KT, 512], BF16, kind="ExternalInput")
    bout = nc.dram_tensor("bout", [1, VPAD], BF16, kind="ExternalInput")
    sbias = nc.dram_tensor("sbias", [1, BLOC, S], BF16, kind="ExternalInput")

    out_lp = nc.dram_tensor("out_lp", [T, BLOC, V], F32, kind="ExternalOutput")
    out_h = nc.dram_tensor("out_h", [128, B], F32, kind="ExternalOutput")
    out_c = nc.dram_tensor("out_c", [128, B], F32, kind="ExternalOutput")

    with tile.TileContext(nc) as tc:
        with (
            tc.tile_pool(name="persist", bufs=1) as persist,
        ):
            ksb = persist.tile([128, BLOC, KT, S], BF16)         # K'^T[b]: [h, s]
            vsb = persist.tile([128, BLOC, ST, KT, 128], BF16)   # V[b] tiles [s, h]
            wic_sb = persist.tile([128, KT, 4, 128], BF16)
            whh_sb = persist.tile([128, KT, 4, 128], BF16)
            gch_sb = persist.tile([128, 4], F32)
            ones_sb = persist.tile([1, 128], BF16)
            sel_sb = persist.tile([16, BLOC], BF16)
            hmask_sb = persist.tile([128, BLOC, KT * B], BF16)
            sb_sb = persist.tile([1, BLOC, S], BF16)
            # stored H (=2h): [p, mb, kt, m] with m = (t%64)*BLOC + bl
            h2_loc = persist.tile([128, 2, KT, 128], BF16)
            nc.gpsimd.memset(h2_loc[:], 0.0)

            nc.sync.dma_start(wic_sb[:], wic[:])
            nc.sync.dma_start(whh_sb[:], whh[:])
            nc.sync.dma_start(gch_sb[:], gch[:])
            nc.sync.dma_start(sel_sb[:], sel[:])
            nc.sync.dma_start(hmask_sb[:], hmask[:])
            if with_sb:
                nc.sync.dma_start(sb_sb[:], sbias[:])
            nc.gpsimd.memset(ones_sb[:], 1.0)

            # ================= phase 0: K', V projections =================
            with (
                tc.tile_pool(name="p0sbuf", bufs=1) as p0s,
                tc.tile_pool(name="p0psum", bufs=4, space="PSUM") as p0p,
            ):
                encs = p0s.tile([128, ET, BLOC, S], BF16, tag="enc")
                nc.sync.dma_start(encs[:], enc_t[:])
                wkq_sb = p0s.tile([128, ET, KT, 128], BF16, tag="wkq")
                nc.sync.dma_start(wkq_sb[:], wkq[:])
                wv_sb = p0s.tile([128, ET, H], BF16, tag="wv")
                nc.sync.dma_start(wv_sb[:], wv[:])

                for b in range(BLOC):
                    for ht in range(KT):
                        ps = p0p.tile([128, S], F32)
                        for et in range(ET):
                            nc.tensor.matmul(
                                ps[:], wkq_sb[:, et, ht, :], encs[:, et, b, :],
                                start=(et == 0), stop=(et == ET - 1))
                        nc.vector.tensor_copy(ksb[:, b, ht, :], ps[:])
                    for st in range(ST):
                        for hc in range(2):
                            ps = p0p.tile([128, S], F32)
                            for et in range(ET):
                                nc.tensor.matmul(
                                    ps[:],
                                    encs[:, et, b, st * 128:(st + 1) * 128],
                                    wv_sb[:, et, hc * 512:(hc + 1) * 512],
                                    start=(et == 0), stop=(et == ET - 1))
                            nc.vector.tensor_copy(
                                vsb[:, b, st, 4 * hc:4 * hc + 4, :], ps[:])

            # ================= recurrence =================
            with (
                tc.tile_pool(name="state", bufs=3) as state,
                tc.tile_pool(name="work", bufs=3) as work,
                tc.tile_pool(name="ps_g", bufs=1, space="PSUM") as ps_g_pool,
                tc.tile_pool(name="ps_c", bufs=1, space="PSUM") as ps_c_pool,
                tc.tile_pool(name="ps_m", bufs=2, space="PSUM") as ps_m_pool,
                tc.tile_pool(name="agd", bufs=4, space="DRAM") as agd,
            ):
                h_full = state.tile([128, KT, B], BF16, tag="h")
                nc.sync.dma_start(h_full[:], h0t[:])
                c_prev = state.tile([128, B], F32, tag="c")
                nc.sync.dma_start(c_prev[:], c0t[:])
                # mask-select h0 local-batch columns for step-0 scores
                h0_loc = state.tile([128, KT * BLOC], BF16, tag="h0l")
                for bl in range(BLOC):
                    tmp = work.tile([128, KT, B], BF16, tag="hm")
                    nc.vector.tensor_mul(
                        tmp[:], h_full[:],
                        hmask_sb[:, bl, :].rearrange("p (k b) -> p k b", k=KT))
                    with nc.allow_low_precision(
                            reason="one-hot mask select, exact"):
                        nc.vector.tensor_reduce(
                            h0_loc[:].rearrange("p (k b) -> p k b", b=BLOC)
                            [:, :, bl:bl + 1],
                            tmp[:], axis=mybir.AxisListType.X, op=OP.add)

                def hcol(t, kt, bl):
                    if t == 0:
                        return h0_loc[:, kt * BLOC + bl:kt * BLOC + bl + 1]
                    tp = t - 1
                    m = (tp % 64) * BLOC + bl
                    return h2_loc[:, tp // 64, kt, m:m + 1]

                for t in range(steps):
                    # ---- W_hh part of gates (overlaps attention+AG) ----
                    ps_g = []
                    for g in range(4):
                        pg = ps_g_pool.tile([128, B], F32, tag=f"g{g}")
                        ps_g.append(pg)
                        for kt in range(KT):
                            nc.tensor.matmul(
                                pg[:], whh_sb[:, kt, g, :], h_full[:, kt, :],
                                start=(kt == 0), stop=False)

                    # ---- store local-batch h_t into h2_loc[t-1] (h2 of step t-1) ----
                    if t > 0:
                        tp = t - 1
                        for bl in range(BLOC):
                            tmp = work.tile([128, KT, B], BF16, tag="hm")
                            nc.vector.tensor_mul(
                                tmp[:], h_full[:],
                                hmask_sb[:, bl, :].rearrange(
                                    "p (k b) -> p k b", k=KT))
                            m = (tp % 64) * BLOC + bl
                            with nc.allow_low_precision(
                                    reason="one-hot mask select, exact"):
                                nc.vector.tensor_reduce(
                                    h2_loc[:, tp // 64, :, m:m + 1], tmp[:],
                                    axis=mybir.AxisListType.X, op=OP.add)

                    # ---- attention (local batches) ----
                    wt_sb = work.tile([128, ST * BLOC], BF16, tag="wt")
                    ps_wt = ps_m_pool.tile([128, ST * BLOC], F32, tag="m")
                    for bl in range(BLOC):
                        ps_row = ps_m_pool.tile([1, S], F32, tag="m")
                        for kt in range(KT):
                            nc.tensor.matmul(
                                ps_row[:], hcol(t, kt, bl), ksb[:, bl, kt, :],
                                start=(kt == 0), stop=(kt == KT - 1 and
                                                       not with_sb))
                        if with_sb:
                            nc.tensor.matmul(ps_row[:], ones_sb[:1, :1],
                                             sb_sb[:, bl, :],
                                             start=False, stop=True)
                        # softmax (no max-subtraction; scores are small)
                        w_row = work.tile([1, S], F32, tag="wrow")
                        ssum = work.tile([1, 1], F32, tag="ssum")
                        nc.scalar.activation(w_row[:], ps_row[:], AF.Exp,
                                             accum_out=ssum[:])
                        rsum = work.tile([1, 1], F32, tag="rsum")
                        nc.vector.reciprocal(rsum[:], ssum[:])
                        # transpose w (and fold 1/sum): [1,S] -> [S(4x128), 1]
                        for st in range(ST):
                            nc.tensor.matmul(
                                ps_wt[:, (st * BLOC + bl):(st * BLOC + bl) + 1],
                                w_row[:, st * 128:(st + 1) * 128], rsum[:],
                                start=True, stop=True)
                    nc.vector.tensor_copy(wt_sb[:], ps_wt[:])

                    # ---- ctx^T = sum_s V[b][s,h] * w[b,s] ----
                    ps_ctx = ps_c_pool.tile([128, KT * BLOC], F32, tag="ctx")
                    for bl in range(BLOC):
                        for ht in range(KT):
                            mm = ht * BLOC + bl
                            for st in range(ST):
                                nc.tensor.matmul(
                                    ps_ctx[:, mm:mm + 1],
                                    vsb[:, bl, st, ht, :],
                                    wt_sb[:, (st * BLOC + bl):
                                          (st * BLOC + bl) + 1],
                                    start=(st == 0), stop=(st == ST - 1))
                    ctx_loc = work.tile([128, KT * BLOC], BF16, tag="ctxl")
                    nc.vector.tensor_copy(ctx_loc[:], ps_ctx[:])

                    # ---- AllGather ctx ----
                    ag_in1 = agd.tile([128, KT * BLOC], BF16, tag="agi1")
                    nc.sync.dma_start(ag_in1[:], ctx_loc[:])
                    ag_out1 = agd.tile([NCORES, 128, KT, BLOC], BF16, tag="ago1")
                    nc.gpsimd.collective_compute(
                        "AllGather", OP.bypass,
                        replica_groups=[list(range(NCORES))],
                        ins=[ag_in1[:]], outs=[ag_out1[:]])
                    ctx_full = work.tile([128, KT, B], BF16, tag="ctxf")
                    nc.sync.dma_start(
                        ctx_full[:].rearrange("p k (r b) -> p k r b", r=NCORES),
                        ag_out1[:].rearrange("r p k b -> p k r b"))

                    # ---- W_ic part of gates ----
                    for g in range(4):
                        for kt in range(KT):
                            nc.tensor.matmul(
                                ps_g[g][:], wic_sb[:, kt, g, :],
                                ctx_full[:, kt, :],
                                start=False, stop=(kt == KT - 1))

                    # ---- elementwise LSTM cell (doubled-state form) ----
                    t_i = work.tile([128, B], F32, tag="ti")
                    t_f = work.tile([128, B], F32, tag="tf")
                    t_g = work.tile([128, B], F32, tag="tg")
                    t_o = work.tile([128, B], F32, tag="to")
                    nc.scalar.activation(t_i[:], ps_g[0][:], AF.Tanh,
                                         bias=gch_sb[:, 0:1], scale=0.5)
                    nc.scalar.activation(t_f[:], ps_g[1][:], AF.Tanh,
                                         bias=gch_sb[:, 1:2], scale=0.5)
                    nc.scalar.activation(t_g[:], ps_g[2][:], AF.Tanh,
                                         bias=gch_sb[:, 2:3], scale=1.0)
                    nc.scalar.activation(t_o[:], ps_g[3][:], AF.Tanh,
                                         bias=gch_sb[:, 3:4], scale=0.5)
                    a_t = work.tile([128, B], F32, tag="at")
                    nc.vector.scalar_tensor_tensor(
                        a_t[:], t_f[:], 1.0, c_prev[:], op0=OP.add, op1=OP.mult)
                    b_t = work.tile([128, B], F32, tag="bt")
                    nc.vector.scalar_tensor_tensor(
                        b_t[:], t_i[:], 1.0, t_g[:], op0=OP.add, op1=OP.mult)
                    c_new = state.tile([128, B], F32, tag="c")
                    nc.vector.scalar_tensor_tensor(
                        c_new[:], a_t[:], 0.5, b_t[:], op0=OP.mult, op1=OP.add)
                    tc2 = work.tile([128, B], F32, tag="tc2")
                    nc.scalar.activation(tc2[:], c_new[:], AF.Tanh, scale=0.5)
                    h_f32 = work.tile([128, B], F32, tag="hf32")
                    nc.vector.scalar_tensor_tensor(
                        h_f32[:], t_o[:], 1.0, tc2[:], op0=OP.add, op1=OP.mult)
                    h_bf = work.tile([128, B], BF16, tag="hbf")
                    nc.vector.tensor_copy(h_bf[:], h_f32[:])

                    # ---- AllGather h ----
                    ag_in2 = agd.tile([128, B], BF16, tag="agi2")
                    nc.sync.dma_start(ag_in2[:], h_bf[:])
                    ag_out2 = agd.tile([NCORES, 128, B], BF16, tag="ago2")
                    nc.gpsimd.collective_compute(
                        "AllGather", OP.bypass,
                        replica_groups=[list(range(NCORES))],
                        ins=[ag_in2[:]], outs=[ag_out2[:]])
                    h_full = state.tile([128, KT, B], BF16, tag="h")
                    nc.sync.dma_start(
                        h_full[:], ag_out2[:].rearrange("r p b -> p r b"))

                    c_prev = c_new
                    if t == steps - 1:
                        nc.sync.dma_start(out_c[:], c_new[:])
                        nc.sync.dma_start(out_h[:], h_f32[:])

                # store the final h (output of the last step) into h2_loc[T-1]
                tp = steps - 1
                for bl in range(BLOC):
                    tmp = work.tile([128, KT, B], BF16, tag="hm")
                    nc.vector.tensor_mul(
                        tmp[:], h_full[:],
                        hmask_sb[:, bl, :].rearrange("p (k b) -> p k b", k=KT))
                    m = (tp % 64) * BLOC + bl
                    with nc.allow_low_precision(
                            reason="one-hot mask select, exact"):
                        nc.vector.tensor_reduce(
                            h2_loc[:, tp // 64, :, m:m + 1], tmp[:],
                            axis=mybir.AxisListType.X, op=OP.add)

            # ================= logits + log_softmax =================
            with (
                tc.tile_pool(name="lg", bufs=3) as lg,
                tc.tile_pool(name="lgbig", bufs=1) as lgbig,
                tc.tile_pool(name="lps", bufs=4, space="PSUM") as lps,
            ):
                if with_bout:
                    bout_sb = lgbig.tile([1, VPAD], BF16, tag="bout")
                    nc.sync.dma_start(bout_sb[:], bout[:])
                lstore = lgbig.tile([128, VPAD], BF16)
                sums = lg.tile([128, VCH], F32, tag="sums")
                for mb in range(2):
                    for vc in range(VCH):
                        wtile = lg.tile([128, KT, 512], BF16, tag="wtile")
                        nc.sync.dma_start(wtile[:], wout[vc])
                        ps = lps.tile([128, 512], F32, tag="lp")
                        for kt in range(KT):
                            nc.tensor.matmul(
                                ps[:], h2_loc[:, mb, kt, :],
                                wtile[:, kt, :],
                                start=(kt == 0), stop=(with_bout is False
                                                       and kt == KT - 1))
                        if with_bout:
                            nc.tensor.matmul(
                                ps[:], ones_sb[:1, :],
                                bout_sb[:, 512 * vc:512 * (vc + 1)],
                                start=False, stop=True)
                        ncc = 512 if vc < VCH - 1 else V - 512 * vc
                        nc.vector.tensor_copy(
                            lstore[:, 512 * vc:512 * vc + ncc], ps[:, :ncc])
                        # logits are small here: exp without max-shift is safe
                        escr = lg.tile([128, 512], F32, tag="escr")
                        nc.scalar.activation(
                            escr[:, :ncc], ps[:, :ncc], AF.Exp,
                            accum_out=sums[:, vc:vc + 1])
                    ssum = lg.tile([128, 1], F32, tag="lsum")
                    nc.vector.tensor_reduce(ssum[:], sums[:],
                                            axis=mybir.AxisListType.X, op=OP.add)
                    lns = lg.tile([128, 1], F32, tag="lns")
                    nc.scalar.activation(lns[:], ssum[:], AF.Ln)
                    nlse = lg.tile([128, 1], F32, tag="nlse")
                    nc.vector.tensor_scalar_mul(nlse[:], lns[:], -1.0)
                    CB = 2048
                    for cb in range(0, V, CB):
                        ncols = min(CB, V - cb)
                        ov = lg.tile([128, CB], F32, tag="ov")
                        nc.scalar.activation(
                            ov[:, :ncols], lstore[:, cb:cb + ncols],
                            AF.Identity, bias=nlse[:])
                        nc.sync.dma_start(
                            out_lp[64 * mb:64 * mb + 64, :, cb:cb + ncols],
                            ov[:, :ncols])

    nc.compile()
    return nc


def _prep(inputs):
    """Host-side weight folding + per-core input arrays."""
    enc = np.asarray(inputs["encoder_outputs"], np.float32)
    h0 = np.asarray(inputs["encoder_h"], np.float32)[0]     # [B,H]
    c0 = np.asarray(inputs["encoder_c"], np.float32)[0]
    emb_tab = np.asarray(inputs["embedding"], np.float32)
    Wq = np.asarray(inputs["Wq"], np.float32)
    bq = np.asarray(inputs["bq"], np.float32)
    Wk = np.asarray(inputs["Wk"], np.float32)
    bk = np.asarray(inputs["bk"], np.float32)
    Wv = np.asarray(inputs["Wv"], np.float32)
    bv = np.asarray(inputs["bv"], np.float32)
    W_ih = np.asarray(inputs["W_ih"], np.float32)
    b_ih = np.asarray(inputs["b_ih"], np.float32)
    W_hh = np.asarray(inputs["W_hh"], np.float32)
    b_hh = np.asarray(inputs["b_hh"], np.float32)
    W_out = np.asarray(inputs["W_out"], np.float32)
    b_out = np.asarray(inputs["b_out"], np.float32)

    scale = 1.0 / np.sqrt(np.float32(H))
    emb = emb_tab[SOS]                                      # [H]
    W_ii, W_ic = W_ih[:, :H], W_ih[:, H:]
    # K' = enc @ M ; M = Wk.T @ Wq * scale * 0.5 (0.5: h stored doubled)
    M = (Wk.T @ Wq) * (scale * 0.5)
    # score bias rows: scale*(enc @ (Wk.T@bq) + bq.bk) (0 when bq==0)
    with_sb = bool(np.any(bq))
    with_bout = bool(np.any(b_out))
    sb_full = scale * (enc @ (Wk.T @ bq) + np.dot(bq, bk))  # [B,S]
    gc = W_ii @ emb + b_ih + b_hh + W_ic @ bv               # [4H]
    W_hh_h = 0.5 * W_hh
    Wout_h = 0.5 * W_out                                    # logits use H=2h

    # shared tensors (bf16 via ml_dtypes)
    import ml_dtypes
    bf = ml_dtypes.bfloat16
    wkq_a = np.ascontiguousarray(
        M.reshape(ET, 128, KT, 128).transpose(1, 0, 2, 3)).astype(bf)
    wv_a = np.ascontiguousarray(
        Wv.T.reshape(ET, 128, H).transpose(1, 0, 2)).astype(bf)
    h0t_a = np.ascontiguousarray(
        (2.0 * h0).T.reshape(KT, 128, B).transpose(1, 0, 2)).astype(bf)
    wout_a = np.zeros((VCH, 128, KT, 512), bf)
    wo = Wout_h.T.astype(np.float32)                        # [H, V]
    wo_pad = np.zeros((H, VPAD), np.float32)
    wo_pad[:, :V] = wo
    wout_a[:] = np.ascontiguousarray(
        wo_pad.reshape(KT, 128, VCH, 512).transpose(2, 1, 0, 3)).astype(bf)
    bout_a = np.full((1, VPAD), -1e30, np.float32)
    bout_a[0, :V] = b_out
    bout_a = bout_a.astype(bf)

    in_maps = []
    for j in range(NCORES):
        bsl = slice(2 * j, 2 * j + 2)
        enc_j = enc[bsl]                                    # [2,S,H]
        enc_t_a = np.ascontiguousarray(
            enc_j.transpose(2, 0, 1).reshape(ET, 128, BLOC, S)
            .transpose(1, 0, 2, 3)).astype(bf)
        rows = np.concatenate(
            [np.arange(g * H + j * 128, g * H + (j + 1) * 128) for g in range(4)])
        wic_j = W_ic[rows]                                  # [4*128, H]
        wic_a = np.ascontiguousarray(
            wic_j.reshape(4, 128, KT, 128).transpose(3, 2, 0, 1)).astype(bf)
        whh_j = W_hh_h[rows]
        whh_a = np.ascontiguousarray(
            whh_j.reshape(4, 128, KT, 128).transpose(3, 2, 0, 1)).astype(bf)
        gc_j = gc[rows].reshape(4, 128).T.copy()            # [128, 4]
        gc_j[:, 0] *= 0.5
        gc_j[:, 1] *= 0.5
        gc_j[:, 3] *= 0.5
        c0_a = np.ascontiguousarray(
            (2.0 * c0[:, j * 128:(j + 1) * 128]).T).astype(np.float32)
        sel_a = np.zeros((16, BLOC), np.float32)
        for bl in range(BLOC):
            sel_a[2 * j + bl, bl] = 1.0
        hm = np.zeros((BLOC, KT, B), np.float32)
        for bl in range(BLOC):
            hm[bl, :, 2 * j + bl] = 1.0
        hmask_a = np.broadcast_to(
            hm.reshape(1, BLOC, KT * B), (128, BLOC, KT * B))
        sb_a = sb_full[bsl].reshape(1, BLOC, S)
        in_maps.append({
            "enc_t": enc_t_a,
            "wkq": wkq_a, "wv": wv_a,
            "wic": wic_a, "whh": whh_a,
            "gch": np.ascontiguousarray(gc_j, np.float32),
            "h0t": h0t_a,
            "c0t": c0_a,
            "sel": np.ascontiguousarray(sel_a).astype(bf),
            "hmask": np.ascontiguousarray(hmask_a).astype(bf),
            "wout": wout_a, "bout": bout_a,
            "sbias": np.ascontiguousarray(sb_a).astype(bf),
        })
    return in_maps, with_sb, with_bout


def kernel(**inputs):
    in_maps, with_sb, with_bout = _prep(inputs)
    key = (T, with_sb, with_bout)
    if key not in _NC_CACHE:
        _NC_CACHE[key] = _build_nc(T, with_sb, with_bout)
    nc = _NC_CACHE[key]
    res = run_bass_kernel_spmd(nc, in_maps, core_ids=list(range(NCORES)))
    lp = np.zeros((B, T, V), np.float32)
    hT = np.zeros((B, H), np.float32)
    cT = np.zeros((B, H), np.float32)
    for j in range(NCORES):
        r = res.results[j]
        o = r["out_lp"]                                     # [T, BLOC, V]
        for bl in range(BLOC):
            lp[2 * j + bl] = o[:, bl, :]
        hT[:, j * 128:(j + 1) * 128] = 0.5 * r["out_h"].T
        cT[:, j * 128:(j + 1) * 128] = 0.5 * r["out_c"].T
    return lp, (hT[None], cT[None])


# revision 14
# speedup vs baseline: 1.1830x; 1.1830x over previous
"""Trainium2 Bass kernel for nn_Decoder (attention + LSTM decoder, T=128 steps).

Sharding (8 NeuronCores, one chip):
- Host folds: K' = enc @ (Wk.T @ Wq) * scale so per-step attention needs no q
  matmul; bk folded away (softmax-invariant), bv folded into gate constants.
- Recurrence: tensor-parallel LSTM (each core owns 128 rows of each of the 4
  gates), batch-parallel attention (2 batches/core). Two AllGathers per step
  (ctx and h). States stored doubled (H=2h, C=2c) so sigmoid comes from the
  Tanh table: sig(x) = 0.5*(tanh(x/2)+1); consumers' weights pre-scaled.
- All rank-dependence is carried by per-core input data (one-hot selectors /
  masks), so one identical program runs SPMD on all 8 cores.
- Output projection + log_softmax: sharded over batch (2/core), W_out streamed
  from HBM in bf16, fused logsumexp.
"""
import numpy as np

import concourse.bacc as bacc
import concourse.mybir as mybir
import concourse.tile as tile
from concourse.bass_utils import run_bass_kernel_spmd

F32 = mybir.dt.float32
BF16 = mybir.dt.bfloat16
AF = mybir.ActivationFunctionType
OP = mybir.AluOpType

V, H, S, B, T = 32000, 1024, 512, 16, 128
SOS = 1
NCORES = 8
BLOC = B // NCORES          # 2 batches per core
KT = H // 128               # 8 h-tiles
ST = S // 128               # 4 s-tiles
ET = H // 128               # 8 e-tiles
VCH = 63                    # v-chunks of 512 (padded)
VPAD = VCH * 512            # 32256

_NC_CACHE = {}


def _build_nc(steps, with_sb, with_bout):
    nc = bacc.Bacc("TRN2", target_bir_lowering=False, debug=False,
                   num_devices=NCORES)

    enc_t = nc.dram_tensor("enc_t", [128, ET, BLOC, S], BF16, kind="ExternalInput")
    wkq = nc.dram_tensor("wkq", [128, ET, KT, 128], BF16, kind="ExternalInput")
    wv = nc.dram_tensor("wv", [128, ET, H], BF16, kind="ExternalInput")
    wic = nc.dram_tensor("wic", [128, KT, 4, 128], BF16, kind="ExternalInput")
    whh = nc.dram_tensor("whh", [128, KT, 4, 128], BF16, kind="ExternalInput")
    gch = nc.dram_tensor("gch", [128, 4], F32, kind="ExternalInput")
    h0t = nc.dram_tensor("h0t", [128, KT, B], BF16, kind="ExternalInput")
    c0t = nc.dram_tensor("c0t", [128, B], F32, kind="ExternalInput")
    sel = nc.dram_tensor("sel", [16, BLOC], BF16, kind="ExternalInput")
    hmask = nc.dram_tensor("hmask", [128, BLOC, KT * B], BF16,
                           kind="ExternalInput")
    wout = nc.dram_tensor("wout", [VCH, 128, KT, 512], BF16, kind="ExternalInput")
    bout = nc.dram_tensor("bout", [1, VPAD], BF16, kind="ExternalInput")
    sbias = nc.dram_tensor("sbias", [1, BLOC, S], BF16, kind="ExternalInput")

    out_lp = nc.dram_tensor("out_lp", [T, BLOC, V], F32, kind="ExternalOutput")
    out_h = nc.dram_tensor("out_h", [128, B], F32, kind="ExternalOutput")
    out_c = nc.dram_tensor("out_c", [128, B], F32, kind="ExternalOutput")

    with tile.TileContext(nc) as tc:
        with (
            tc.tile_pool(name="persist", bufs=1) as persist,
        ):
            ksb = persist.tile([128, BLOC, KT, S], BF16)         # K'^T[b]: [h, s]
            vsb = persist.tile([128, BLOC, ST, KT, 128], BF16)   # V[b] tiles [s, h]
            wic_sb = persist.tile([128, KT, 4, 128], BF16)
            whh_sb = persist.tile([128, KT, 4, 128], BF16)
            gch_sb = persist.tile([128, 4], F32)
            ones_sb = persist.tile([1, 128], BF16)
            sel_sb = persist.tile([16, BLOC], BF16)
            hmask_sb = persist.tile([128, BLOC, KT * B], BF16)
            sb_sb = persist.tile([1, BLOC, S], BF16)
            # stored H (=2h): [p, mb, kt, m] with m = (t%64)*BLOC + bl
            h2_loc = persist.tile([128, 2, KT, 128], BF16)
            nc.gpsimd.memset(h2_loc[:], 0.0)

            nc.sync.dma_start(wic_sb[:], wic[:])
            nc.sync.dma_start(whh_sb[:], whh[:])
            nc.sync.dma_start(gch_sb[:], gch[:])
            nc.sync.dma_start(sel_sb[:], sel[:])
            nc.sync.dma_start(hmask_sb[:], hmask[:])
            if with_sb:
                nc.sync.dma_start(sb_sb[:], sbias[:])
            nc.gpsimd.memset(ones_sb[:], 1.0)

            # ================= phase 0: K', V projections =================
            with (
                tc.tile_pool(name="p0sbuf", bufs=1) as p0s,
                tc.tile_pool(name="p0psum", bufs=4, space="PSUM") as p0p,
            ):
                encs = p0s.tile([128, ET, BLOC, S], BF16, tag="enc")
                nc.sync.dma_start(encs[:], enc_t[:])
                wkq_sb = p0s.tile([128, ET, KT, 128], BF16, tag="wkq")
                nc.sync.dma_start(wkq_sb[:], wkq[:])
                wv_sb = p0s.tile([128, ET, H], BF16, tag="wv")
                nc.sync.dma_start(wv_sb[:], wv[:])

                for b in range(BLOC):
                    for ht in range(KT):
                        ps = p0p.tile([128, S], F32)
                        for et in range(ET):
                            nc.tensor.matmul(
                                ps[:], wkq_sb[:, et, ht, :], encs[:, et, b, :],
                                start=(et == 0), stop=(et == ET - 1))
                        nc.vector.tensor_copy(ksb[:, b, ht, :], ps[:])
                    for st in range(ST):
                        for hc in range(2):
                            ps = p0p.tile([128, S], F32)
                            for et in range(ET):
                                nc.tensor.matmul(
                                    ps[:],
                                    encs[:, et, b, st * 128:(st + 1) * 128],
                                    wv_sb[:, et, hc * 512:(hc + 1) * 512],
                                    start=(et == 0), stop=(et == ET - 1))
                            nc.vector.tensor_copy(
                                vsb[:, b, st, 4 * hc:4 * hc + 4, :], ps[:])

            # ================= recurrence =================
            with (
                tc.tile_pool(name="state", bufs=3) as state,
                tc.tile_pool(name="work", bufs=3) as work,
                tc.tile_pool(name="ps_g", bufs=1, space="PSUM") as ps_g_pool,
                tc.tile_pool(name="ps_c", bufs=1, space="PSUM") as ps_c_pool,
                tc.tile_pool(name="ps_m", bufs=2, space="PSUM") as ps_m_pool,
                tc.tile_pool(name="agd", bufs=4, space="DRAM") as agd,
            ):
                h_full = state.tile([128, KT, B], BF16, tag="h")
                nc.sync.dma_start(h_full[:], h0t[:])
                c_prev = state.tile([128, B], F32, tag="c")
                nc.sync.dma_start(c_prev[:], c0t[:])
                # mask-select h0 local-batch columns for step-0 scores
                h0_loc = state.tile([128, KT * BLOC], BF16, tag="h0l")
                for bl in range(BLOC):
                    tmp = work.tile([128, KT, B], BF16, tag="hm")
                    nc.vector.tensor_mul(
                        tmp[:], h_full[:],
                        hmask_sb[:, bl, :].rearrange("p (k b) -> p k b", k=KT))
                    with nc.allow_low_precision(
                            reason="one-hot mask select, exact"):
                        nc.vector.tensor_reduce(
                            h0_loc[:].rearrange("p (k b) -> p k b", b=BLOC)
                            [:, :, bl:bl + 1],
                            tmp[:], axis=mybir.AxisListType.X, op=OP.add)

                def hcol(t, kt, bl):
                    if t == 0:
                        return h0_loc[:, kt * BLOC + bl:kt * BLOC + bl + 1]
                    tp = t - 1
                    m = (tp % 64) * BLOC + bl
                    return h2_loc[:, tp // 64, kt, m:m + 1]

                for t in range(steps):
                    # ---- W_hh part of gates (overlaps attention+AG) ----
                    ps_g = []
                    for g in range(4):
                        pg = ps_g_pool.tile([128, B], F32, tag=f"g{g}")
                        ps_g.append(pg)
                        for kt in range(KT):
                            nc.tensor.matmul(
                                pg[:], whh_sb[:, kt, g, :], h_full[:, kt, :],
                                start=(kt == 0), stop=False)

                    # ---- store local-batch h_t into h2_loc[t-1] (h2 of step t-1) ----
                    if t > 0:
                        tp = t - 1
                        for bl in range(BLOC):
                            tmp = work.tile([128, KT, B], BF16, tag="hm")
                            nc.vector.tensor_mul(
                                tmp[:], h_full[:],
                                hmask_sb[:, bl, :].rearrange(
                                    "p (k b) -> p k b", k=KT))
                            m = (tp % 64) * BLOC + bl
                            with nc.allow_low_precision(
                                    reason="one-hot mask select, exact"):
                                nc.vector.tensor_reduce(
                                    h2_loc[:, tp // 64, :, m:m + 1], tmp[:],
                                    axis=mybir.AxisListType.X, op=OP.add)

                    # ---- attention (local batches) ----
                    wt_sb = work.tile([128, ST * BLOC], BF16, tag="wt")
                    ps_wt = ps_m_pool.tile([128, ST * BLOC], F32, tag="m")
                    for bl in range(BLOC):
                        ps_row = ps_m_pool.tile([1, S], F32, tag="m")
                        for kt in range(KT):
                            nc.tensor.matmul(
                                ps_row[:], hcol(t, kt, bl), ksb[:, bl, kt, :],
                                start=(kt == 0), stop=(kt == KT - 1 and
                                                       not with_sb))
                        if with_sb:
                            nc.tensor.matmul(ps_row[:], ones_sb[:1, :1],
                                             sb_sb[:, bl, :],
                                             start=False, stop=True)
                        # softmax (no max-subtraction; scores are small)
                        w_row = work.tile([1, S], F32, tag="wrow")
                        ssum = work.tile([1, 1], F32, tag="ssum")
                        nc.scalar.activation(w_row[:], ps_row[:], AF.Exp,
                                             accum_out=ssum[:])
                        rsum = work.tile([1, 1], F32, tag="rsum")
                        nc.vector.reciprocal(rsum[:], ssum[:])
                        # transpose w (and fold 1/sum): [1,S] -> [S(4x128), 1]
                        for st in range(ST):
                            nc.tensor.matmul(
                                ps_wt[:, (st * BLOC + bl):(st * BLOC + bl) + 1],
                                w_row[:, st * 128:(st + 1) * 128], rsum[:],
                                start=True, stop=True)
                    nc.vector.tensor_copy(wt_sb[:], ps_wt[:])

                    # ---- ctx^T = sum_s V[b][s,h] * w[b,s] ----
                    ps_ctx = ps_c_pool.tile([128, KT * BLOC], F32, tag="ctx")
                    for bl in range(BLOC):
                        for ht in range(KT):
                            mm = ht * BLOC + bl
                            for st in range(ST):
                                nc.tensor.matmul(
                                    ps_ctx[:, mm:mm + 1],
                                    vsb[:, bl, st, ht, :],
                                    wt_sb[:, (st * BLOC + bl):
                                          (st * BLOC + bl) + 1],
                                    start=(st == 0), stop=(st == ST - 1))
                    ctx_loc = work.tile([128, KT * BLOC], BF16, tag="ctxl")
                    nc.vector.tensor_copy(ctx_loc[:], ps_ctx[:])

                    # ---- AllGather ctx ----
                    ag_in1 = agd.tile([128, KT * BLOC], BF16, tag="agi1")
                    nc.sync.dma_start(ag_in1[:], ctx_loc[:])
                    ag_out1 = agd.tile([NCORES, 128, KT, BLOC], BF16, tag="ago1")
                    nc.gpsimd.collective_compute(
                        "AllGather", OP.bypass,
                        replica_groups=[list(range(NCORES))],
                        ins=[ag_in1[:]], outs=[ag_out1[:]])
                    ctx_tmp = work.tile([128, NCORES, KT, BLOC], BF16,
                                        tag="ctxt")
                    nc.sync.dma_start(
                        ctx_tmp[:], ag_out1[:].rearrange("r p k b -> p r k b"))
                    ctx_full = work.tile([128, KT, B], BF16, tag="ctxf")
                    nc.vector.tensor_copy(
                        ctx_full[:].rearrange("p k (r b) -> p r k b", r=NCORES),
                        ctx_tmp[:])

                    # ---- W_ic part of gates ----
                    for g in range(4):
                        for kt in range(KT):
                            nc.tensor.matmul(
                                ps_g[g][:], wic_sb[:, kt, g, :],
                                ctx_full[:, kt, :],
                                start=False, stop=(kt == KT - 1))

                    # ---- elementwise LSTM cell (doubled-state form) ----
                    t_i = work.tile([128, B], F32, tag="ti")
                    t_f = work.tile([128, B], F32, tag="tf")
                    t_g = work.tile([128, B], F32, tag="tg")
                    t_o = work.tile([128, B], F32, tag="to")
                    nc.scalar.activation(t_i[:], ps_g[0][:], AF.Tanh,
                                         bias=gch_sb[:, 0:1], scale=0.5)
                    nc.scalar.activation(t_f[:], ps_g[1][:], AF.Tanh,
                                         bias=gch_sb[:, 1:2], scale=0.5)
                    nc.scalar.activation(t_g[:], ps_g[2][:], AF.Tanh,
                                         bias=gch_sb[:, 2:3], scale=1.0)
                    nc.scalar.activation(t_o[:], ps_g[3][:], AF.Tanh,
                                         bias=gch_sb[:, 3:4], scale=0.5)
                    a_t = work.tile([128, B], F32, tag="at")
                    nc.vector.scalar_tensor_tensor(
                        a_t[:], t_f[:], 1.0, c_prev[:], op0=OP.add, op1=OP.mult)
                    b_t = work.tile([128, B], F32, tag="bt")
                    nc.vector.scalar_tensor_tensor(
                        b_t[:], t_i[:], 1.0, t_g[:], op0=OP.add, op1=OP.mult)
                    c_new = state.tile([128, B], F32, tag="c")
                    nc.vector.scalar_tensor_tensor(
                        c_new[:], a_t[:], 0.5, b_t[:], op0=OP.mult, op1=OP.add)
                    tc2 = work.tile([128, B], F32, tag="tc2")
                    nc.scalar.activation(tc2[:], c_new[:], AF.Tanh, scale=0.5)
                    h_f32 = work.tile([128, B], F32, tag="hf32")
                    nc.vector.scalar_tensor_tensor(
                        h_f32[:], t_o[:], 1.0, tc2[:], op0=OP.add, op1=OP.mult)
                    h_bf = work.tile([128, B], BF16, tag="hbf")
                    nc.vector.tensor_copy(h_bf[:], h_f32[:])

                    # ---- AllGather h ----
                    ag_in2 = agd.tile([128, B], BF16, tag="agi2")
                    nc.sync.dma_start(ag_in2[:], h_bf[:])
                    ag_out2 = agd.tile([NCORES, 128, B], BF16, tag="ago2")
                    nc.gpsimd.collective_compute(
                        "AllGather", OP.bypass,
                        replica_groups=[list(range(NCORES))],
                        ins=[ag_in2[:]], outs=[ag_out2[:]])
                    h_full = state.tile([128, KT, B], BF16, tag="h")
                    nc.sync.dma_start(
                        h_full[:], ag_out2[:].rearrange("r p b -> p r b"))

                    c_prev = c_new
                    if t == steps - 1:
                        nc.sync.dma_start(out_c[:], c_new[:])
                        nc.sync.dma_start(out_h[:], h_f32[:])

                # store the final h (output of the last step) into h2_loc[T-1]
                tp = steps - 1
                for bl in range(BLOC):
                    tmp = work.tile([128, KT, B], BF16, tag="hm")
                    nc.vector.tensor_mul(
                        tmp[:], h_full[:],
                        hmask_sb[:, bl, :].rearrange("p (k b) -> p k b", k=KT))
                    m = (tp % 64) * BLOC + bl
                    with nc.allow_low_precision(
                            reason="one-hot mask select, exact"):
                        nc.vector.tensor_reduce(
                            h2_loc[:, tp // 64, :, m:m + 1], tmp[:],
                            axis=mybir.AxisListType.X, op=OP.add)

            # ================= logits + log_softmax =================
            with (
                tc.tile_pool(name="lg", bufs=3) as lg,
                tc.tile_pool(name="lgbig", bufs=1) as lgbig,
                tc.tile_pool(name="lps", bufs=4, space="PSUM") as lps,
            ):
                if with_bout:
                    bout_sb = lgbig.tile([1, VPAD], BF16, tag="bout")
                    nc.sync.dma_start(bout_sb[:], bout[:])
                lstore = lgbig.tile([128, VPAD], BF16)
                sums = lg.tile([128, VCH], F32, tag="sums")
                for mb in range(2):
                    for vc in range(VCH):
                        wtile = lg.tile([128, KT, 512], BF16, tag="wtile")
                        nc.sync.dma_start(wtile[:], wout[vc])
                        ps = lps.tile([128, 512], F32, tag="lp")
                        for kt in range(KT):
                            nc.tensor.matmul(
                                ps[:], h2_loc[:, mb, kt, :],
                                wtile[:, kt, :],
                                start=(kt == 0), stop=(with_bout is False
                                                       and kt == KT - 1))
                        if with_bout:
                            nc.tensor.matmul(
                                ps[:], ones_sb[:1, :],
                                bout_sb[:, 512 * vc:512 * (vc + 1)],
                                start=False, stop=True)
                        ncc = 512 if vc < VCH - 1 else V - 512 * vc
                        nc.vector.tensor_copy(
                            lstore[:, 512 * vc:512 * vc + ncc], ps[:, :ncc])
                        # logits are small here: exp without max-shift is safe
                        escr = lg.tile([128, 512], F32, tag="escr")
                        nc.scalar.activation(
                            escr[:, :ncc], ps[:, :ncc], AF.Exp,
                            accum_out=sums[:, vc:vc + 1])
                    ssum = lg.tile([128, 1], F32, tag="lsum")
                    nc.vector.tensor_reduce(ssum[:], sums[:],
                                            axis=mybir.AxisListType.X, op=OP.add)
                    lns = lg.tile([128, 1], F32, tag="lns")
                    nc.scalar.activation(lns[:], ssum[:], AF.Ln)
                    nlse = lg.tile([128, 1], F32, tag="nlse")
                    nc.vector.tensor_scalar_mul(nlse[:], lns[:], -1.0)
                    CB = 2048
                    for cb in range(0, V, CB):
                        ncols = min(CB, V - cb)
                        ov = lg.tile([128, CB], F32, tag="ov")
                        nc.scalar.activation(
                            ov[:, :ncols], lstore[:, cb:cb + ncols],
                            AF.Identity, bias=nlse[:])
                        nc.sync.dma_start(
                            out_lp[64 * mb:64 * mb + 64, :, cb:cb + ncols],
                            ov[:, :ncols])

    nc.compile()
    return nc


def _prep(inputs):
    """Host-side weight folding + per-core input arrays."""
    enc = np.asarray(inputs["encoder_outputs"], np.float32)
    h0 = np.asarray(inputs["encoder_h"], np.float32)[0]     # [B,H]
    c0 = np.asarray(inputs["encoder_c"], np.float32)[0]
    emb_tab = np.asarray(inputs["embedding"], np.float32)
    Wq = np.asarray(inputs["Wq"], np.float32)
    bq = np.asarray(inputs["bq"], np.float32)
    Wk = np.asarray(inputs["Wk"], np.float32)
    bk = np.asarray(inputs["bk"], np.float32)
    Wv = np.asarray(inputs["Wv"], np.float32)
    bv = np.asarray(inputs["bv"], np.float32)
    W_ih = np.asarray(inputs["W_ih"], np.float32)
    b_ih = np.asarray(inputs["b_ih"], np.float32)
    W_hh = np.asarray(inputs["W_hh"], np.float32)
    b_hh = np.asarray(inputs["b_hh"], np.float32)
    W_out = np.asarray(inputs["W_out"], np.float32)
    b_out = np.asarray(inputs["b_out"], np.float32)

    scale = 1.0 / np.sqrt(np.float32(H))
    emb = emb_tab[SOS]                                      # [H]
    W_ii, W_ic = W_ih[:, :H], W_ih[:, H:]
    # K' = enc @ M ; M = Wk.T @ Wq * scale * 0.5 (0.5: h stored doubled)
    M = (Wk.T @ Wq) * (scale * 0.5)
    # score bias rows: scale*(enc @ (Wk.T@bq) + bq.bk) (0 when bq==0)
    with_sb = bool(np.any(bq))
    with_bout = bool(np.any(b_out))
    sb_full = scale * (enc @ (Wk.T @ bq) + np.dot(bq, bk))  # [B,S]
    gc = W_ii @ emb + b_ih + b_hh + W_ic @ bv               # [4H]
    W_hh_h = 0.5 * W_hh
    Wout_h = 0.5 * W_out                                    # logits use H=2h

    # shared tensors (bf16 via ml_dtypes)
    import ml_dtypes
    bf = ml_dtypes.bfloat16
    wkq_a = np.ascontiguousarray(
        M.reshape(ET, 128, KT, 128).transpose(1, 0, 2, 3)).astype(bf)
    wv_a = np.ascontiguousarray(
        Wv.T.reshape(ET, 128, H).transpose(1, 0, 2)).astype(bf)
    h0t_a = np.ascontiguousarray(
        (2.0 * h0).T.reshape(KT, 128, B).transpose(1, 0, 2)).astype(bf)
    wout_a = np.zeros((VCH, 128, KT, 512), bf)
    wo = Wout_h.T.astype(np.float32)                        # [H, V]
    wo_pad = np.zeros((H, VPAD), np.float32)
    wo_pad[:, :V] = wo
    wout_a[:] = np.ascontiguousarray(
        wo_pad.reshape(KT, 128, VCH, 512).transpose(2, 1, 0, 3)).astype(bf)
    bout_a = np.full((1, VPAD), -1e30, np.float32)
    bout_a[0, :V] = b_out
    bout_a = bout_a.astype(bf)

    in_maps = []
    for j in range(NCORES):
        bsl = slice(2 * j, 2 * j + 2)
        enc_j = enc[bsl]                                    # [2,S,H]
        enc_t_a = np.ascontiguousarray(
            enc_j.transpose(2, 0, 1).reshape(ET, 128, BLOC, S)
            .transpose(1, 0, 2, 3)).astype(bf)
        rows = np.concatenate(
            [np.arange(g * H + j * 128, g * H + (j + 1) * 128) for g in range(4)])
        wic_j = W_ic[rows]                                  # [4*128, H]
        wic_a = np.ascontiguousarray(
            wic_j.reshape(4, 128, KT, 128).transpose(3, 2, 0, 1)).astype(bf)
        whh_j = W_hh_h[rows]
        whh_a = np.ascontiguousarray(
            whh_j.reshape(4, 128, KT, 128).transpose(3, 2, 0, 1)).astype(bf)
        gc_j = gc[rows].reshape(4, 128).T.copy()            # [128, 4]
        gc_j[:, 0] *= 0.5
        gc_j[:, 1] *= 0.5
        gc_j[:, 3] *= 0.5
        c0_a = np.ascontiguousarray(
            (2.0 * c0[:, j * 128:(j + 1) * 128]).T).astype(np.float32)
        sel_a = np.zeros((16, BLOC), np.float32)
        for bl in range(BLOC):
            sel_a[2 * j + bl, bl] = 1.0
        hm = np.zeros((BLOC, KT, B), np.float32)
        for bl in range(BLOC):
            hm[bl, :, 2 * j + bl] = 1.0
        hmask_a = np.broadcast_to(
            hm.reshape(1, BLOC, KT * B), (128, BLOC, KT * B))
        sb_a = sb_full[bsl].reshape(1, BLOC, S)
        in_maps.append({
            "enc_t": enc_t_a,
            "wkq": wkq_a, "wv": wv_a,
            "wic": wic_a, "whh": whh_a,
            "gch": np.ascontiguousarray(gc_j, np.float32),
            "h0t": h0t_a,
            "c0t": c0_a,
            "sel": np.ascontiguousarray(sel_a).astype(bf),
            "hmask": np.ascontiguousarray(hmask_a).astype(bf),
            "wout": wout_a, "bout": bout_a,
            "sbias": np.ascontiguousarray(sb_a).astype(bf),
        })
    return in_maps, with_sb, with_bout


def kernel(**inputs):
    in_maps, with_sb, with_bout = _prep(inputs)
    key = (T, with_sb, with_bout)
    if key not in _NC_CACHE:
        _NC_CACHE[key] = _build_nc(T, with_sb, with_bout)
    nc = _NC_CACHE[key]
    res = run_bass_kernel_spmd(nc, in_maps, core_ids=list(range(NCORES)))
    lp = np.zeros((B, T, V), np.float32)
    hT = np.zeros((B, H), np.float32)
    cT = np.zeros((B, H), np.float32)
    for j in range(NCORES):
        r = res.results[j]
        o = r["out_lp"]                                     # [T, BLOC, V]
        for bl in range(BLOC):
            lp[2 * j + bl] = o[:, bl, :]
        hT[:, j * 128:(j + 1) * 128] = 0.5 * r["out_h"].T
        cT[:, j * 128:(j + 1) * 128] = 0.5 * r["out_c"].T
    return lp, (hT[None], cT[None])


# revision 15
# speedup vs baseline: 1.1981x; 1.0127x over previous
"""Trainium2 Bass kernel for nn_Decoder (attention + LSTM decoder, T=128 steps).

Sharding (8 NeuronCores, one chip):
- Host folds: K' = enc @ (Wk.T @ Wq) * scale so per-step attention needs no q
  matmul; bk folded away (softmax-invariant), bv folded into gate constants.
- Recurrence: tensor-parallel LSTM (each core owns 128 rows of each of the 4
  gates), batch-parallel attention (2 batches/core). Two AllGathers per step
  (ctx and h). States stored doubled (H=2h, C=2c) so sigmoid comes from the
  Tanh table: sig(x) = 0.5*(tanh(x/2)+1); consumers' weights pre-scaled.
- All rank-dependence is carried by per-core input data (one-hot selectors /
  masks), so one identical program runs SPMD on all 8 cores.
- Output projection + log_softmax: sharded over batch (2/core), W_out streamed
  from HBM in bf16, fused logsumexp.
"""
import numpy as np

import concourse.bacc as bacc
import concourse.mybir as mybir
import concourse.tile as tile
from concourse.bass_utils import run_bass_kernel_spmd

F32 = mybir.dt.float32
BF16 = mybir.dt.bfloat16
AF = mybir.ActivationFunctionType
OP = mybir.AluOpType

V, H, S, B, T = 32000, 1024, 512, 16, 128
SOS = 1
NCORES = 8
BLOC = B // NCORES          # 2 batches per core
KT = H // 128               # 8 h-tiles
ST = S // 128               # 4 s-tiles
ET = H // 128               # 8 e-tiles
VCH = 63                    # v-chunks of 512 (padded)
VPAD = VCH * 512            # 32256

_NC_CACHE = {}


def _build_nc(steps, with_sb, with_bout):
    nc = bacc.Bacc("TRN2", target_bir_lowering=False, debug=False,
                   num_devices=NCORES)

    enc_t = nc.dram_tensor("enc_t", [128, ET, BLOC, S], BF16, kind="ExternalInput")
    wkq = nc.dram_tensor("wkq", [128, ET, KT, 128], BF16, kind="ExternalInput")
    wv = nc.dram_tensor("wv", [128, ET, H], BF16, kind="ExternalInput")
    wic = nc.dram_tensor("wic", [128, KT, 4, 128], BF16, kind="ExternalInput")
    whh = nc.dram_tensor("whh", [128, KT, 4, 128], BF16, kind="ExternalInput")
    gch = nc.dram_tensor("gch", [128, 4], F32, kind="ExternalInput")
    h0t = nc.dram_tensor("h0t", [128, KT, B], BF16, kind="ExternalInput")
    c0t = nc.dram_tensor("c0t", [128, B], F32, kind="ExternalInput")
    sel = nc.dram_tensor("sel", [16, BLOC], BF16, kind="ExternalInput")
    hmask = nc.dram_tensor("hmask", [128, BLOC, KT * B], BF16,
                           kind="ExternalInput")
    wout = nc.dram_tensor("wout", [VCH, 128, KT, 512], BF16, kind="ExternalInput")
    bout = nc.dram_tensor("bout", [1, VPAD], BF16, kind="ExternalInput")
    sbias = nc.dram_tensor("sbias", [1, BLOC, S], BF16, kind="ExternalInput")

    out_lp = nc.dram_tensor("out_lp", [T, BLOC, V], F32, kind="ExternalOutput")
    out_h = nc.dram_tensor("out_h", [128, B], F32, kind="ExternalOutput")
    out_c = nc.dram_tensor("out_c", [128, B], F32, kind="ExternalOutput")

    with tile.TileContext(nc) as tc:
        with (
            tc.tile_pool(name="persist", bufs=1) as persist,
        ):
            ksb = persist.tile([128, BLOC, KT, S], BF16)         # K'^T[b]: [h, s]
            vsb = persist.tile([128, BLOC, ST, KT, 128], BF16)   # V[b] tiles [s, h]
            wic_sb = persist.tile([128, KT, 4, 128], BF16)
            whh_sb = persist.tile([128, KT, 4, 128], BF16)
            gch_sb = persist.tile([128, 4], F32)
            ones_sb = persist.tile([1, 128], BF16)
            sel_sb = persist.tile([16, BLOC], BF16)
            hmask_sb = persist.tile([128, BLOC, KT * B], BF16)
            sb_sb = persist.tile([1, BLOC, S], BF16)
            # stored H (=2h): [p, mb, kt, m] with m = (t%64)*BLOC + bl
            h2_loc = persist.tile([128, 2, KT, 128], BF16)
            nc.gpsimd.memset(h2_loc[:], 0.0)

            nc.sync.dma_start(wic_sb[:], wic[:])
            nc.sync.dma_start(whh_sb[:], whh[:])
            nc.sync.dma_start(gch_sb[:], gch[:])
            nc.sync.dma_start(sel_sb[:], sel[:])
            nc.sync.dma_start(hmask_sb[:], hmask[:])
            if with_sb:
                nc.sync.dma_start(sb_sb[:], sbias[:])
            nc.gpsimd.memset(ones_sb[:], 1.0)

            # ================= phase 0: K', V projections =================
            with (
                tc.tile_pool(name="p0sbuf", bufs=1) as p0s,
                tc.tile_pool(name="p0psum", bufs=4, space="PSUM") as p0p,
            ):
                encs = p0s.tile([128, ET, BLOC, S], BF16, tag="enc")
                nc.sync.dma_start(encs[:], enc_t[:])
                wkq_sb = p0s.tile([128, ET, KT, 128], BF16, tag="wkq")
                nc.sync.dma_start(wkq_sb[:], wkq[:])
                wv_sb = p0s.tile([128, ET, H], BF16, tag="wv")
                nc.sync.dma_start(wv_sb[:], wv[:])

                for b in range(BLOC):
                    for ht in range(KT):
                        ps = p0p.tile([128, S], F32)
                        for et in range(ET):
                            nc.tensor.matmul(
                                ps[:], wkq_sb[:, et, ht, :], encs[:, et, b, :],
                                start=(et == 0), stop=(et == ET - 1))
                        nc.vector.tensor_copy(ksb[:, b, ht, :], ps[:])
                    for st in range(ST):
                        for hc in range(2):
                            ps = p0p.tile([128, S], F32)
                            for et in range(ET):
                                nc.tensor.matmul(
                                    ps[:],
                                    encs[:, et, b, st * 128:(st + 1) * 128],
                                    wv_sb[:, et, hc * 512:(hc + 1) * 512],
                                    start=(et == 0), stop=(et == ET - 1))
                            nc.vector.tensor_copy(
                                vsb[:, b, st, 4 * hc:4 * hc + 4, :], ps[:])

            # ================= recurrence =================
            with (
                tc.tile_pool(name="state", bufs=3) as state,
                tc.tile_pool(name="work", bufs=3) as work,
                tc.tile_pool(name="ps_g", bufs=1, space="PSUM") as ps_g_pool,
                tc.tile_pool(name="ps_c", bufs=1, space="PSUM") as ps_c_pool,
                tc.tile_pool(name="ps_m", bufs=2, space="PSUM") as ps_m_pool,
                tc.tile_pool(name="ps_w", bufs=1, space="PSUM") as ps_w_pool,
                tc.tile_pool(name="agd", bufs=4, space="DRAM") as agd,
            ):
                h_full = state.tile([128, KT, B], BF16, tag="h")
                nc.sync.dma_start(h_full[:], h0t[:])
                c_prev = state.tile([128, B], F32, tag="c")
                nc.sync.dma_start(c_prev[:], c0t[:])
                # mask-select h0 local-batch columns for step-0 scores
                h0_loc = state.tile([128, KT * BLOC], BF16, tag="h0l")
                for bl in range(BLOC):
                    tmp = work.tile([128, KT, B], BF16, tag="hm")
                    nc.vector.tensor_mul(
                        tmp[:], h_full[:],
                        hmask_sb[:, bl, :].rearrange("p (k b) -> p k b", k=KT))
                    with nc.allow_low_precision(
                            reason="one-hot mask select, exact"):
                        nc.vector.tensor_reduce(
                            h0_loc[:].rearrange("p (k b) -> p k b", b=BLOC)
                            [:, :, bl:bl + 1],
                            tmp[:], axis=mybir.AxisListType.X, op=OP.add)

                def hcol(t, kt, bl):
                    if t == 0:
                        return h0_loc[:, kt * BLOC + bl:kt * BLOC + bl + 1]
                    tp = t - 1
                    m = (tp % 64) * BLOC + bl
                    return h2_loc[:, tp // 64, kt, m:m + 1]

                for t in range(steps):
                    # ---- W_hh part of gates (overlaps attention+AG) ----
                    ps_g = []
                    for g in range(4):
                        pg = ps_g_pool.tile([128, B], F32, tag=f"g{g}")
                        ps_g.append(pg)
                        for kt in range(KT):
                            nc.tensor.matmul(
                                pg[:], whh_sb[:, kt, g, :], h_full[:, kt, :],
                                start=(kt == 0), stop=False)

                    # ---- store local-batch h_t into h2_loc[t-1] (h2 of step t-1) ----
                    if t > 0:
                        tp = t - 1
                        for bl in range(BLOC):
                            tmp = work.tile([128, KT, B], BF16, tag="hm")
                            nc.vector.tensor_mul(
                                tmp[:], h_full[:],
                                hmask_sb[:, bl, :].rearrange(
                                    "p (k b) -> p k b", k=KT))
                            m = (tp % 64) * BLOC + bl
                            with nc.allow_low_precision(
                                    reason="one-hot mask select, exact"):
                                nc.vector.tensor_reduce(
                                    h2_loc[:, tp // 64, :, m:m + 1], tmp[:],
                                    axis=mybir.AxisListType.X, op=OP.add)

                    # ---- attention (local batches) ----
                    wt_sb = work.tile([128, ST * BLOC], BF16, tag="wt")
                    ps_wt = ps_m_pool.tile([128, ST * BLOC], F32, tag="m")
                    for bl in range(BLOC):
                        ps_row = ps_m_pool.tile([1, S], F32, tag="m")
                        for kt in range(KT):
                            nc.tensor.matmul(
                                ps_row[:], hcol(t, kt, bl), ksb[:, bl, kt, :],
                                start=(kt == 0), stop=(kt == KT - 1 and
                                                       not with_sb))
                        if with_sb:
                            nc.tensor.matmul(ps_row[:], ones_sb[:1, :1],
                                             sb_sb[:, bl, :],
                                             start=False, stop=True)
                        # softmax (no max-subtraction; scores are small)
                        w_row = work.tile([1, S], F32, tag="wrow")
                        ssum = work.tile([1, 1], F32, tag="ssum")
                        nc.scalar.activation(w_row[:], ps_row[:], AF.Exp,
                                             accum_out=ssum[:])
                        rsum = work.tile([1, 1], F32, tag="rsum")
                        nc.vector.reciprocal(rsum[:], ssum[:])
                        # transpose w (and fold 1/sum): [1,S] -> [S(4x128), 1]
                        for st in range(ST):
                            nc.tensor.matmul(
                                ps_wt[:, (st * BLOC + bl):(st * BLOC + bl) + 1],
                                w_row[:, st * 128:(st + 1) * 128], rsum[:],
                                start=True, stop=True)
                    nc.vector.tensor_copy(wt_sb[:], ps_wt[:])

                    # ---- ctx^T = sum_s V[b][s,h] * w[b,s] ----
                    ps_ctx = ps_c_pool.tile([128, KT * BLOC], F32, tag="ctx")
                    for bl in range(BLOC):
                        for ht in range(KT):
                            mm = ht * BLOC + bl
                            for st in range(ST):
                                nc.tensor.matmul(
                                    ps_ctx[:, mm:mm + 1],
                                    vsb[:, bl, st, ht, :],
                                    wt_sb[:, (st * BLOC + bl):
                                          (st * BLOC + bl) + 1],
                                    start=(st == 0), stop=(st == ST - 1))
                    ctx_loc = work.tile([128, KT * BLOC], BF16, tag="ctxl")
                    nc.vector.tensor_copy(ctx_loc[:], ps_ctx[:])

                    # ---- AllGather ctx ----
                    ag_in1 = agd.tile([128, KT * BLOC], BF16, tag="agi1")
                    nc.sync.dma_start(ag_in1[:], ctx_loc[:])
                    ag_out1 = agd.tile([NCORES, 128, KT, BLOC], BF16, tag="ago1")
                    nc.gpsimd.collective_compute(
                        "AllGather", OP.bypass,
                        replica_groups=[list(range(NCORES))],
                        ins=[ag_in1[:]], outs=[ag_out1[:]])
                    # HAM-warming dummies: keep PE busy through the AG gap
                    NW = 40
                    ps_warm = ps_w_pool.tile([16, 512], F32, tag="warm")
                    for _ in range(NW):
                        nc.tensor.matmul(ps_warm[:], ctx_loc[:, :16],
                                         ksb[:, 0, 0, :], start=True, stop=True)
                    ctx_tmp = work.tile([128, NCORES, KT, BLOC], BF16,
                                        tag="ctxt")
                    nc.sync.dma_start(
                        ctx_tmp[:], ag_out1[:].rearrange("r p k b -> p r k b"))
                    ctx_full = work.tile([128, KT, B], BF16, tag="ctxf")
                    nc.vector.tensor_copy(
                        ctx_full[:].rearrange("p k (r b) -> p r k b", r=NCORES),
                        ctx_tmp[:])

                    # ---- W_ic part of gates ----
                    for g in range(4):
                        for kt in range(KT):
                            nc.tensor.matmul(
                                ps_g[g][:], wic_sb[:, kt, g, :],
                                ctx_full[:, kt, :],
                                start=False, stop=(kt == KT - 1))

                    # ---- elementwise LSTM cell (doubled-state form) ----
                    t_i = work.tile([128, B], F32, tag="ti")
                    t_f = work.tile([128, B], F32, tag="tf")
                    t_g = work.tile([128, B], F32, tag="tg")
                    t_o = work.tile([128, B], F32, tag="to")
                    nc.scalar.activation(t_i[:], ps_g[0][:], AF.Tanh,
                                         bias=gch_sb[:, 0:1], scale=0.5)
                    nc.scalar.activation(t_f[:], ps_g[1][:], AF.Tanh,
                                         bias=gch_sb[:, 1:2], scale=0.5)
                    nc.scalar.activation(t_g[:], ps_g[2][:], AF.Tanh,
                                         bias=gch_sb[:, 2:3], scale=1.0)
                    nc.scalar.activation(t_o[:], ps_g[3][:], AF.Tanh,
                                         bias=gch_sb[:, 3:4], scale=0.5)
                    a_t = work.tile([128, B], F32, tag="at")
                    nc.vector.scalar_tensor_tensor(
                        a_t[:], t_f[:], 1.0, c_prev[:], op0=OP.add, op1=OP.mult)
                    b_t = work.tile([128, B], F32, tag="bt")
                    nc.vector.scalar_tensor_tensor(
                        b_t[:], t_i[:], 1.0, t_g[:], op0=OP.add, op1=OP.mult)
                    c_new = state.tile([128, B], F32, tag="c")
                    nc.vector.scalar_tensor_tensor(
                        c_new[:], a_t[:], 0.5, b_t[:], op0=OP.mult, op1=OP.add)
                    tc2 = work.tile([128, B], F32, tag="tc2")
                    nc.scalar.activation(tc2[:], c_new[:], AF.Tanh, scale=0.5)
                    h_f32 = work.tile([128, B], F32, tag="hf32")
                    nc.vector.scalar_tensor_tensor(
                        h_f32[:], t_o[:], 1.0, tc2[:], op0=OP.add, op1=OP.mult)
                    h_bf = work.tile([128, B], BF16, tag="hbf")
                    nc.vector.tensor_copy(h_bf[:], h_f32[:])

                    # ---- AllGather h ----
                    ag_in2 = agd.tile([128, B], BF16, tag="agi2")
                    nc.sync.dma_start(ag_in2[:], h_bf[:])
                    ag_out2 = agd.tile([NCORES, 128, B], BF16, tag="ago2")
                    nc.gpsimd.collective_compute(
                        "AllGather", OP.bypass,
                        replica_groups=[list(range(NCORES))],
                        ins=[ag_in2[:]], outs=[ag_out2[:]])
                    ps_warm2 = ps_w_pool.tile([16, 512], F32, tag="warm")
                    for _ in range(NW):
                        nc.tensor.matmul(ps_warm2[:], h_bf[:, :16],
                                         ksb[:, 0, 0, :], start=True, stop=True)
                    h_full = state.tile([128, KT, B], BF16, tag="h")
                    nc.sync.dma_start(
                        h_full[:], ag_out2[:].rearrange("r p b -> p r b"))

                    c_prev = c_new
                    if t == steps - 1:
                        nc.sync.dma_start(out_c[:], c_new[:])
                        nc.sync.dma_start(out_h[:], h_f32[:])

                # store the final h (output of the last step) into h2_loc[T-1]
                tp = steps - 1
                for bl in range(BLOC):
                    tmp = work.tile([128, KT, B], BF16, tag="hm")
                    nc.vector.tensor_mul(
                        tmp[:], h_full[:],
                        hmask_sb[:, bl, :].rearrange("p (k b) -> p k b", k=KT))
                    m = (tp % 64) * BLOC + bl
                    with nc.allow_low_precision(
                            reason="one-hot mask select, exact"):
                        nc.vector.tensor_reduce(
                            h2_loc[:, tp // 64, :, m:m + 1], tmp[:],
                            axis=mybir.AxisListType.X, op=OP.add)

            # ================= logits + log_softmax =================
            with (
                tc.tile_pool(name="lg", bufs=3) as lg,
                tc.tile_pool(name="lgbig", bufs=1) as lgbig,
                tc.tile_pool(name="lps", bufs=4, space="PSUM") as lps,
            ):
                if with_bout:
                    bout_sb = lgbig.tile([1, VPAD], BF16, tag="bout")
                    nc.sync.dma_start(bout_sb[:], bout[:])
                lstore = lgbig.tile([128, VPAD], BF16)
                sums = lg.tile([128, VCH], F32, tag="sums")
                for mb in range(2):
                    for vc in range(VCH):
                        wtile = lg.tile([128, KT, 512], BF16, tag="wtile")
                        nc.sync.dma_start(wtile[:], wout[vc])
                        ps = lps.tile([128, 512], F32, tag="lp")
                        for kt in range(KT):
                            nc.tensor.matmul(
                                ps[:], h2_loc[:, mb, kt, :],
                                wtile[:, kt, :],
                                start=(kt == 0), stop=(with_bout is False
                                                       and kt == KT - 1))
                        if with_bout:
                            nc.tensor.matmul(
                                ps[:], ones_sb[:1, :],
                                bout_sb[:, 512 * vc:512 * (vc + 1)],
                                start=False, stop=True)
                        ncc = 512 if vc < VCH - 1 else V - 512 * vc
                        nc.vector.tensor_copy(
                            lstore[:, 512 * vc:512 * vc + ncc], ps[:, :ncc])
                        # logits are small here: exp without max-shift is safe
                        escr = lg.tile([128, 512], F32, tag="escr")
                        nc.scalar.activation(
                            escr[:, :ncc], ps[:, :ncc], AF.Exp,
                            accum_out=sums[:, vc:vc + 1])
                    ssum = lg.tile([128, 1], F32, tag="lsum")
                    nc.vector.tensor_reduce(ssum[:], sums[:],
                                            axis=mybir.AxisListType.X, op=OP.add)
                    lns = lg.tile([128, 1], F32, tag="lns")
                    nc.scalar.activation(lns[:], ssum[:], AF.Ln)
                    nlse = lg.tile([128, 1], F32, tag="nlse")
                    nc.vector.tensor_scalar_mul(nlse[:], lns[:], -1.0)
                    CB = 2048
                    for cb in range(0, V, CB):
                        ncols = min(CB, V - cb)
                        ov = lg.tile([128, CB], F32, tag="ov")
                        nc.scalar.activation(
                            ov[:, :ncols], lstore[:, cb:cb + ncols],
                            AF.Identity, bias=nlse[:])
                        nc.sync.dma_start(
                            out_lp[64 * mb:64 * mb + 64, :, cb:cb + ncols],
                            ov[:, :ncols])

    nc.compile()
    return nc


def _prep(inputs):
    """Host-side weight folding + per-core input arrays."""
    enc = np.asarray(inputs["encoder_outputs"], np.float32)
    h0 = np.asarray(inputs["encoder_h"], np.float32)[0]     # [B,H]
    c0 = np.asarray(inputs["encoder_c"], np.float32)[0]
    emb_tab = np.asarray(inputs["embedding"], np.float32)
    Wq = np.asarray(inputs["Wq"], np.float32)
    bq = np.asarray(inputs["bq"], np.float32)
    Wk = np.asarray(inputs["Wk"], np.float32)
    bk = np.asarray(inputs["bk"], np.float32)
    Wv = np.asarray(inputs["Wv"], np.float32)
    bv = np.asarray(inputs["bv"], np.float32)
    W_ih = np.asarray(inputs["W_ih"], np.float32)
    b_ih = np.asarray(inputs["b_ih"], np.float32)
    W_hh = np.asarray(inputs["W_hh"], np.float32)
    b_hh = np.asarray(inputs["b_hh"], np.float32)
    W_out = np.asarray(inputs["W_out"], np.float32)
    b_out = np.asarray(inputs["b_out"], np.float32)

    scale = 1.0 / np.sqrt(np.float32(H))
    emb = emb_tab[SOS]                                      # [H]
    W_ii, W_ic = W_ih[:, :H], W_ih[:, H:]
    # K' = enc @ M ; M = Wk.T @ Wq * scale * 0.5 (0.5: h stored doubled)
    M = (Wk.T @ Wq) * (scale * 0.5)
    # score bias rows: scale*(enc @ (Wk.T@bq) + bq.bk) (0 when bq==0)
    with_sb = bool(np.any(bq))
    with_bout = bool(np.any(b_out))
    sb_full = scale * (enc @ (Wk.T @ bq) + np.dot(bq, bk))  # [B,S]
    gc = W_ii @ emb + b_ih + b_hh + W_ic @ bv               # [4H]
    W_hh_h = 0.5 * W_hh
    Wout_h = 0.5 * W_out                                    # logits use H=2h

    # shared tensors (bf16 via ml_dtypes)
    import ml_dtypes
    bf = ml_dtypes.bfloat16
    wkq_a = np.ascontiguousarray(
        M.reshape(ET, 128, KT, 128).transpose(1, 0, 2, 3)).astype(bf)
    wv_a = np.ascontiguousarray(
        Wv.T.reshape(ET, 128, H).transpose(1, 0, 2)).astype(bf)
    h0t_a = np.ascontiguousarray(
        (2.0 * h0).T.reshape(KT, 128, B).transpose(1, 0, 2)).astype(bf)
    wout_a = np.zeros((VCH, 128, KT, 512), bf)
    wo = Wout_h.T.astype(np.float32)                        # [H, V]
    wo_pad = np.zeros((H, VPAD), np.float32)
    wo_pad[:, :V] = wo
    wout_a[:] = np.ascontiguousarray(
        wo_pad.reshape(KT, 128, VCH, 512).transpose(2, 1, 0, 3)).astype(bf)
    bout_a = np.full((1, VPAD), -1e30, np.float32)
    bout_a[0, :V] = b_out
    bout_a = bout_a.astype(bf)

    in_maps = []
    for j in range(NCORES):
        bsl = slice(2 * j, 2 * j + 2)
        enc_j = enc[bsl]                                    # [2,S,H]
        enc_t_a = np.ascontiguousarray(
            enc_j.transpose(2, 0, 1).reshape(ET, 128, BLOC, S)
            .transpose(1, 0, 2, 3)).astype(bf)
        rows = np.concatenate(
            [np.arange(g * H + j * 128, g * H + (j + 1) * 128) for g in range(4)])
        wic_j = W_ic[rows]                                  # [4*128, H]
        wic_a = np.ascontiguousarray(
            wic_j.reshape(4, 128, KT, 128).transpose(3, 2, 0, 1)).astype(bf)
        whh_j = W_hh_h[rows]
        whh_a = np.ascontiguousarray(
            whh_j.reshape(4, 128, KT, 128).transpose(3, 2, 0, 1)).astype(bf)
        gc_j = gc[rows].reshape(4, 128).T.copy()            # [128, 4]
        gc_j[:, 0] *= 0.5
        gc_j[:, 1] *= 0.5
        gc_j[:, 3] *= 0.5
        c0_a = np.ascontiguousarray(
            (2.0 * c0[:, j * 128:(j + 1) * 128]).T).astype(np.float32)
        sel_a = np.zeros((16, BLOC), np.float32)
        for bl in range(BLOC):
            sel_a[2 * j + bl, bl] = 1.0
        hm = np.zeros((BLOC, KT, B), np.float32)
        for bl in range(BLOC):
            hm[bl, :, 2 * j + bl] = 1.0
        hmask_a = np.broadcast_to(
            hm.reshape(1, BLOC, KT * B), (128, BLOC, KT * B))
        sb_a = sb_full[bsl].reshape(1, BLOC, S)
        in_maps.append({
            "enc_t": enc_t_a,
            "wkq": wkq_a, "wv": wv_a,
            "wic": wic_a, "whh": whh_a,
            "gch": np.ascontiguousarray(gc_j, np.float32),
            "h0t": h0t_a,
            "c0t": c0_a,
            "sel": np.ascontiguousarray(sel_a).astype(bf),
            "hmask": np.ascontiguousarray(hmask_a).astype(bf),
            "wout": wout_a, "bout": bout_a,
            "sbias": np.ascontiguousarray(sb_a).astype(bf),
        })
    return in_maps, with_sb, with_bout


def kernel(**inputs):
    in_maps, with_sb, with_bout = _prep(inputs)
    key = (T, with_sb, with_bout)
    if key not in _NC_CACHE:
        _NC_CACHE[key] = _build_nc(T, with_sb, with_bout)
    nc = _NC_CACHE[key]
    res = run_bass_kernel_spmd(nc, in_maps, core_ids=list(range(NCORES)))
    lp = np.zeros((B, T, V), np.float32)
    hT = np.zeros((B, H), np.float32)
    cT = np.zeros((B, H), np.float32)
    for j in range(NCORES):
        r = res.results[j]
        o = r["out_lp"]                                     # [T, BLOC, V]
        for bl in range(BLOC):
            lp[2 * j + bl] = o[:, bl, :]
        hT[:, j * 128:(j + 1) * 128] = 0.5 * r["out_h"].T
        cT[:, j * 128:(j + 1) * 128] = 0.5 * r["out_c"].T
    return lp, (hT[None], cT[None])
